# revision 28
# baseline (speedup 1.0000x reference)
"""Trainium2 Bass kernel for the gated equivariant tensor-product layer.

Math (per node z, MUL=64):
  x0 = feats[:, :64], x1[u,i] = feats[:, 64+3u+i], a0 = attrs[:,0], a1 = attrs[:,1:4]
  out0 = ALPHA*( (x0*a0) @ W1 + C*(sum_i x1_i*a1_i) @ W2 )          # [N,128] = s|g
  out1_i = ALPHA*C*( (x0*a1_i) @ W3 + (x1_i*a0) @ W4 )              # [N,64] per i
  out = [ silu(s) | sigmoid(g)[w]*out1_i[w] at col 64+3w+i ]

Design v5 (engine-balanced; all elementwise on DVE at fp16 2x-mode):
 - HOST layouts (layout/dtype only): featsT [384,n] = [T1|T2|T3] with
   T1=[x0;x1_0], T2=[x1_1;x1_2], T3=[x0;x0]; m1112 [128,n] = [a1_1;a1_2]
   row-replicated; arow [2,n] = (a0, a1_0) scalars.
 - Per chunk: gpsimd partition_broadcast builds A0/A10 [128,n] fp16; DVE
   forms 5 product tiles in 3 ops (3D APs + free-dim stride-0 broadcast):
     P12 = [T1|T2] * bcast(A0)    -> [t0;t4_0 | t4_1;t4_2]
     P45 = [T2|T3] * bcast(M1112) -> [dt_1;dt_2 | t3_1;t3_2]
     P3  =  T1     * A10          -> [t3_0;dt_0]
 - 7 logical fp16 matmuls (zero/blockdiag/fused-column lhsT):
     B1 = LA'P1+LB'P3+LC'P4 = [s;g];  PSX0 = LD'P1+LE'P3 = [o1_0;junk]
     PSX1 = LF'P2+LG'P5 = [o1_1;o1_2]
 - ACT: sigmoid(g) twice -> U=[sg;sg]; one DVE gating op with in1 =
   free-dim-broadcast U covers all 3 gated tiles (junk half overwritten
   by ACT Silu(s) written straight into the output tile).

Sharding: pure data parallelism over nodes, 8 cores x 25000 nodes
(padded to 25600 = 25 chunks of 1024 per core).
"""

import sys
import numpy as np

sys.path.insert(0, "/opt/trn_rl_repo")

MUL = 64
C3 = 1.0 / np.sqrt(3.0)
ALPHA = 1.0 / np.sqrt(MUL * 1 * 2)

N_CORES = 8
N_PER = 25000
N_PAD = 25600
CHUNK = 1024
HALF = CHUNK // 2
N_CHUNKS = N_PAD // CHUNK
P = 128

_BUILT = None


def _build_nc():
    import concourse.bacc as bacc
    import concourse.mybir as mybir
    from concourse.tile import TileContext
    from concourse.dve_ops import AFFINE_MUL_REDUCE

    f32 = mybir.dt.float32
    f16 = mybir.dt.float16
    MULT = mybir.AluOpType.mult
    AF = mybir.ActivationFunctionType

    nc = bacc.Bacc("TRN2", target_bir_lowering=False, debug=False)

    featsT_d = nc.declare_dram_parameter("featsT", [384, N_PAD], f16, isOutput=False)
    # amix rows: 0:128 = [a1_1; a1_2] each replicated x64
    amix_d = nc.declare_dram_parameter("amix", [128, N_PAD], f16, isOutput=False)
    arow_d = nc.declare_dram_parameter("arow", [2, N_PAD], f16, isOutput=False)
    w1_d = nc.declare_dram_parameter("W1", [64, 128], f32, isOutput=False)
    w2_d = nc.declare_dram_parameter("W2", [64, 128], f32, isOutput=False)
    w3_d = nc.declare_dram_parameter("W3", [64, 64], f32, isOutput=False)
    w4_d = nc.declare_dram_parameter("W4", [64, 64], f32, isOutput=False)
    outT_d = nc.declare_dram_parameter("outT", [256, N_PAD], f16, isOutput=True)

    with TileContext(nc) as tc:
        wpool = tc.alloc_tile_pool(name="wpool", bufs=1)
        ft = tc.alloc_tile_pool(name="ft", bufs=5)
        amp = tc.alloc_tile_pool(name="amp", bufs=5)
        a0p = tc.alloc_tile_pool(name="a0p", bufs=5)
        s12p = tc.alloc_tile_pool(name="s12p", bufs=2)
        s45p = tc.alloc_tile_pool(name="s45p", bufs=2)
        s3p = tc.alloc_tile_pool(name="s3p", bufs=2)
        up = tc.alloc_tile_pool(name="up", bufs=2)
        otp = tc.alloc_tile_pool(name="otp", bufs=3)
        osp = tc.alloc_tile_pool(name="osp", bufs=3)
        psb = tc.alloc_tile_pool(name="psb", bufs=2, space="PSUM")
        psx = tc.alloc_tile_pool(name="psx", bufs=1, space="PSUM")

        # ---- weights (once): build 7 lhsT tiles [128,128] f16 ----
        # W1' = ALPHA*W1, W2' = ALPHA*C3*W2, W3' = ALPHA*C3*W3, W4' = ALPHA*C3*W4
        wtmp = wpool.tile([P, 128], f32, tag="wtmp")
        nc.sync.dma_start(wtmp[0:64, :], w1_d[:, :])
        nc.sync.dma_start(wtmp[64:128, :], w2_d[:, :])
        nc.vector.tensor_scalar_mul(wtmp[0:64, :], wtmp[0:64, :], float(ALPHA))
        nc.vector.tensor_scalar_mul(wtmp[64:128, :], wtmp[64:128, :], float(ALPHA * C3))
        wtmp2 = wpool.tile([P, 64], f32, tag="wtmp2")
        nc.sync.dma_start(wtmp2[0:64, :], w3_d[:, :])
        nc.sync.dma_start(wtmp2[64:128, :], w4_d[:, :])
        nc.vector.tensor_scalar_mul(wtmp2[:, :], wtmp2[:, :], float(ALPHA * C3))

        LA = wpool.tile([P, 128], f16, tag="LA")  # [W1'; 0]
        LB = wpool.tile([P, 128], f16, tag="LB")  # [0; W2']
        LC = wpool.tile([P, 128], f16, tag="LC")  # [W2'; W2']
        LD = wpool.tile([P, 128], f16, tag="LD")  # [[0|W1'c64]; [W4'|0]]
        LE = wpool.tile([P, 128], f16, tag="LE")  # [[W3'|0]; 0]
        LF = wpool.tile([P, 128], f16, tag="LF")  # blockdiag(W4', W4')
        LG = wpool.tile([P, 128], f16, tag="LG")  # blockdiag(W3', W3')
        for L in (LA, LB, LD, LE, LF, LG):
            nc.vector.memzero(L[:, :])
        nc.scalar.copy(LA[0:64, :], wtmp[0:64, :])
        nc.scalar.copy(LB[64:128, :], wtmp[64:128, :])
        nc.scalar.copy(LC[0:64, :], wtmp[64:128, :])
        nc.scalar.copy(LC[64:128, :], wtmp[64:128, :])
        nc.scalar.copy(LD[64:128, 0:64], wtmp2[64:128, :])
        nc.scalar.copy(LD[0:64, 64:128], wtmp[0:64, 0:64])
        nc.scalar.copy(LE[0:64, 0:64], wtmp2[0:64, :])
        nc.scalar.copy(LF[0:64, 0:64], wtmp2[64:128, :])
        nc.scalar.copy(LF[64:128, 64:128], wtmp2[64:128, :])
        nc.scalar.copy(LG[0:64, 0:64], wtmp2[0:64, :])
        nc.scalar.copy(LG[64:128, 64:128], wtmp2[0:64, :])

        # whole a0/a1_0 rows resident on partition 0 (102KB); one DMA
        ARall = wpool.tile([1, 2, N_PAD], f16, tag="ARall")
        nc.gpsimd.dma_start(ARall[0:1, :, :], arow_d[:, :].unsqueeze(0))

        def prep_dma(ch):
            z0 = ch * CHUNK
            F = ft.tile([P, 3, CHUNK], f16, tag="F")
            nc.sync.dma_start(
                F[:], featsT_d[:, z0 : z0 + CHUNK].rearrange("(t p) n -> p t n", p=P)
            )
            AM = amp.tile([P, CHUNK], f16, tag="AM")
            nc.gpsimd.dma_start(AM[:, :], amix_d[:, z0 : z0 + CHUNK])
            A0 = a0p.tile([P, 2, CHUNK], f16, tag="A0")
            nc.gpsimd.partition_broadcast(A0[:, 0], ARall[0:1, 0, z0 : z0 + CHUNK])
            nc.gpsimd.partition_broadcast(A0[:, 1], ARall[0:1, 1, z0 : z0 + CHUNK])
            return F, AM, A0

        def prep_prod(ch, tiles):
            F, AM, A0 = tiles
            P12 = s12p.tile([P, 2, CHUNK], f16, tag="P12")
            P45 = s45p.tile([P, 2, CHUNK], f16, tag="P45")
            P3 = s3p.tile([P, CHUNK], f16, tag="P3")
            a0b = A0[:, 0].unsqueeze(1).to_broadcast((P, 2, CHUNK))
            m12b = AM[:, :].unsqueeze(1).to_broadcast((P, 2, CHUNK))
            nc.vector.tensor_tensor(P12[:, :], F[:, 0:2], a0b, MULT)
            nc.vector.tensor_tensor(P45[:, :], F[:, 1:3], m12b, MULT)
            nc.vector.tensor_tensor(P3[:, :], F[:, 0], A0[:, 1], MULT)
            return P12, P45, P3

        def crunch_mm(ch, tiles):
            P12, P45, P3 = tiles
            B1 = psb.tile([P, CHUNK], f32, tag="B1")  # [s; g]
            PSX = psx.tile([P, 2, CHUNK], f32, tag="PSX")  # [o1_0;junk | o1_1;o1_2]
            plan = [
                (LA, B1, None, P12, 0, True, False),
                (LB, B1, None, P3, None, False, False),
                (LC, B1, None, P45, 0, False, True),
                (LD, PSX, 0, P12, 0, True, False),
                (LE, PSX, 0, P3, None, False, True),
                (LF, PSX, 1, P12, 1, True, False),
                (LG, PSX, 1, P45, 1, False, True),
            ]
            for L, dst, dcol, src, scol, st, sp in plan:
                for h in range(2):
                    hs = slice(h * HALF, (h + 1) * HALF)
                    d = dst[:, hs] if dcol is None else dst[:, dcol, hs]
                    s = src[:, hs] if scol is None else src[:, scol, hs]
                    nc.tensor.matmul(d, L[:, :], s, start=st, stop=sp)

            # tanh in the matmul stage: U is ready before the gate stage.
            # sigmoid(g) = 0.5*tanh(g/2)+0.5 -- tanh and silu share one ACT
            # table set (sigmoid does not); the affine folds into the gating.
            U = up.tile([P, CHUNK], f16, tag="U")  # [th; th], th = tanh(g/2)
            nc.scalar.activation(U[0:64, :], B1[64:128, :], AF.Tanh, scale=0.5)
            nc.scalar.activation(U[64:128, :], B1[64:128, :], AF.Tanh, scale=0.5)
            return B1, PSX, U

        def crunch_gate(ch, tiles):
            B1, PSX, U = tiles
            z0 = ch * CHUNK
            OT = otp.tile([P, 2, CHUNK], f16, tag="OT")
            OTS = osp.tile([64, CHUNK], f16, tag="OTS")
            ub = U[:, :].unsqueeze(1).to_broadcast((P, 2, CHUNK))
            # OT = (th*0.5 + 0.5) * PSX = sigmoid(g) * [o1_0;junk | o1_1;o1_2]
            nc.vector._custom_dve(
                AFFINE_MUL_REDUCE, out=OT[:, :], in0=ub, in1=PSX[:, :],
                s0=0.5, s1=0.5,
            )
            # silu into its own tile so it doesn't serialize behind the gating
            nc.scalar.activation(OTS[:, :], B1[0:64, :], AF.Silu)

            # outT rows: 0:64 = OT[0:64,0], 64:128 = OTS, 128:256 = OT[:,1]
            nc.sync.dma_start(outT_d[0:64, z0 : z0 + CHUNK], OT[0:64, 0])
            nc.scalar.dma_start(outT_d[64:128, z0 : z0 + CHUNK], OTS[:, :])
            nc.sync.dma_start(outT_d[128:256, z0 : z0 + CHUNK], OT[:, 1])

        # 5-stage pipeline: dma(ch) -> products(ch-2) -> matmuls(ch-3)
        # -> gate+store(ch-4). Products precede AMR in the DVE queue.
        raw = {}
        prod = {}
        mmres = {}
        for ch in range(N_CHUNKS + 4):
            if ch - 4 in mmres:
                crunch_gate(ch - 4, mmres.pop(ch - 4))
            if ch < N_CHUNKS:
                raw[ch] = prep_dma(ch)
            if ch - 2 in raw:
                prod[ch - 2] = prep_prod(ch - 2, raw.pop(ch - 2))
            if ch - 3 in prod:
                mmres[ch - 3] = crunch_mm(ch - 3, prod.pop(ch - 3))

        for pool in (
            psx, psb, osp, otp, up, s3p, s45p, s12p, a0p, amp, ft, wpool
        ):
            pool.release()

    nc.compile()
    return nc


def _get_nc():
    global _BUILT
    if _BUILT is None:
        _BUILT = _build_nc()
    return _BUILT


def _host_prep(node_feats, node_attrs):
    """Feature-major fp16 layouts per core (layout/dtype/replication only)."""
    feats = np.ascontiguousarray(node_feats, dtype=np.float32)
    attrs = np.ascontiguousarray(node_attrs, dtype=np.float32)
    in_maps = []
    for c in range(N_CORES):
        f = feats[c * N_PER : (c + 1) * N_PER]
        a = attrs[c * N_PER : (c + 1) * N_PER].astype(np.float16)
        n = f.shape[0]
        x0 = f[:, :MUL].astype(np.float16).T
        x1 = f[:, MUL:].reshape(n, MUL, 3).astype(np.float16)
        ftT = np.zeros((384, N_PAD), np.float16)
        ftT[0:64, :n] = x0
        ftT[64:128, :n] = x1[:, :, 0].T
        ftT[128:192, :n] = x1[:, :, 1].T
        ftT[192:256, :n] = x1[:, :, 2].T
        ftT[256:320, :n] = x0
        ftT[320:384, :n] = x0
        amix = np.zeros((128, N_PAD), np.float16)
        amix[0:64, :n] = a[:, 2][None, :]  # a1_1
        amix[64:128, :n] = a[:, 3][None, :]  # a1_2
        arow = np.zeros((2, N_PAD), np.float16)
        arow[0, :n] = a[:, 0]  # a0
        arow[1, :n] = a[:, 1]  # a1_0
        in_maps.append({"featsT": ftT, "amix": amix, "arow": arow})
    return in_maps


def kernel(node_feats, node_attrs, W1, W2, W3, W4):
    from concourse.bass_utils import run_bass_kernel_spmd

    nc = _get_nc()
    in_maps = _host_prep(node_feats, node_attrs)
    for im in in_maps:
        im["W1"] = np.ascontiguousarray(W1, np.float32)
        im["W2"] = np.ascontiguousarray(W2, np.float32)
        im["W3"] = np.ascontiguousarray(W3, np.float32)
        im["W4"] = np.ascontiguousarray(W4, np.float32)

    res = run_bass_kernel_spmd(nc, in_maps, list(range(N_CORES)))
    global LAST_RESULT
    LAST_RESULT = res

    outs = []
    for c in range(N_CORES):
        oT = res.results[c]["outT"][:, :N_PER].astype(np.float32)  # [256, n]
        n = oT.shape[1]
        out = np.empty((n, 256), np.float32)
        # outT rows: 0:64 = gated o1_0, 64:128 = silu(s),
        #            128:192 = gated o1_1, 192:256 = gated o1_2
        out[:, :MUL] = oT[64:128, :].T
        g = np.stack([oT[0:64], oT[128:192], oT[192:256]], axis=0)  # [3, 64, n]
        out[:, MUL:] = g.transpose(2, 1, 0).reshape(n, 192)
        outs.append(out)
    return np.concatenate(outs, axis=0)


LAST_RESULT = None


# revision 29
# speedup vs baseline: 1.1210x; 1.1210x over previous
"""Trainium2 Bass kernel for the gated equivariant tensor-product layer.

Math (per node z, MUL=64):
  x0 = feats[:, :64], x1[u,i] = feats[:, 64+3u+i], a0 = attrs[:,0], a1 = attrs[:,1:4]
  out0 = ALPHA*( (x0*a0) @ W1 + C*(sum_i x1_i*a1_i) @ W2 )          # [N,128] = s|g
  out1_i = ALPHA*C*( (x0*a1_i) @ W3 + (x1_i*a0) @ W4 )              # [N,64] per i
  out = [ silu(s) | sigmoid(g)[w]*out1_i[w] at col 64+3w+i ]

Design v5 (engine-balanced; all elementwise on DVE at fp16 2x-mode):
 - HOST layouts (layout/dtype only): featsT [384,n] = [T1|T2|T3] with
   T1=[x0;x1_0], T2=[x1_1;x1_2], T3=[x0;x0]; m1112 [128,n] = [a1_1;a1_2]
   row-replicated; arow [2,n] = (a0, a1_0) scalars.
 - Per chunk: gpsimd partition_broadcast builds A0/A10 [128,n] fp16; DVE
   forms 5 product tiles in 3 ops (3D APs + free-dim stride-0 broadcast):
     P12 = [T1|T2] * bcast(A0)    -> [t0;t4_0 | t4_1;t4_2]
     P45 = [T2|T3] * bcast(M1112) -> [dt_1;dt_2 | t3_1;t3_2]
     P3  =  T1     * A10          -> [t3_0;dt_0]
 - 7 logical fp16 matmuls (zero/blockdiag/fused-column lhsT):
     B1 = LA'P1+LB'P3+LC'P4 = [s;g];  PSX0 = LD'P1+LE'P3 = [o1_0;junk]
     PSX1 = LF'P2+LG'P5 = [o1_1;o1_2]
 - ACT: sigmoid(g) twice -> U=[sg;sg]; one DVE gating op with in1 =
   free-dim-broadcast U covers all 3 gated tiles (junk half overwritten
   by ACT Silu(s) written straight into the output tile).

Sharding: pure data parallelism over nodes, 8 cores x 25000 nodes
(padded to 25600 = 25 chunks of 1024 per core).
"""

import sys
import numpy as np

sys.path.insert(0, "/opt/trn_rl_repo")

MUL = 64
C3 = 1.0 / np.sqrt(3.0)
ALPHA = 1.0 / np.sqrt(MUL * 1 * 2)

N_CORES = 8
N_PER = 25000
N_PAD = 25600
CHUNK = 1024
HALF = CHUNK // 2
N_CHUNKS = N_PAD // CHUNK
P = 128

_BUILT = None


def _build_nc():
    import concourse.bacc as bacc
    import concourse.mybir as mybir
    from concourse.tile import TileContext
    from concourse.dve_ops import AFFINE_MUL_REDUCE

    f32 = mybir.dt.float32
    f16 = mybir.dt.float16
    MULT = mybir.AluOpType.mult
    AF = mybir.ActivationFunctionType

    nc = bacc.Bacc("TRN2", target_bir_lowering=False, debug=False)

    featsT_d = nc.declare_dram_parameter("featsT", [384, N_PAD], f16, isOutput=False)
    # amix rows: 0:128 = a1_0 replicated, 128:256 = [a1_1; a1_2] repl x64
    amix_d = nc.declare_dram_parameter("amix", [256, N_PAD], f16, isOutput=False)
    arow_d = nc.declare_dram_parameter("arow", [1, N_PAD], f16, isOutput=False)
    w1_d = nc.declare_dram_parameter("W1", [64, 128], f32, isOutput=False)
    w2_d = nc.declare_dram_parameter("W2", [64, 128], f32, isOutput=False)
    w3_d = nc.declare_dram_parameter("W3", [64, 64], f32, isOutput=False)
    w4_d = nc.declare_dram_parameter("W4", [64, 64], f32, isOutput=False)
    outT_d = nc.declare_dram_parameter("outT", [256, N_PAD], f16, isOutput=True)

    with TileContext(nc) as tc:
        wpool = tc.alloc_tile_pool(name="wpool", bufs=1)
        ft = tc.alloc_tile_pool(name="ft", bufs=5)
        amp = tc.alloc_tile_pool(name="amp", bufs=5)
        a0p = tc.alloc_tile_pool(name="a0p", bufs=5)
        s12p = tc.alloc_tile_pool(name="s12p", bufs=2)
        s45p = tc.alloc_tile_pool(name="s45p", bufs=2)
        s3p = tc.alloc_tile_pool(name="s3p", bufs=2)
        up = tc.alloc_tile_pool(name="up", bufs=2)
        otp = tc.alloc_tile_pool(name="otp", bufs=3)
        osp = tc.alloc_tile_pool(name="osp", bufs=3)
        psb = tc.alloc_tile_pool(name="psb", bufs=2, space="PSUM")
        psx = tc.alloc_tile_pool(name="psx", bufs=1, space="PSUM")

        # ---- weights (once): build 7 lhsT tiles [128,128] f16 ----
        # W1' = ALPHA*W1, W2' = ALPHA*C3*W2, W3' = ALPHA*C3*W3, W4' = ALPHA*C3*W4
        wtmp = wpool.tile([P, 128], f32, tag="wtmp")
        nc.sync.dma_start(wtmp[0:64, :], w1_d[:, :])
        nc.sync.dma_start(wtmp[64:128, :], w2_d[:, :])
        nc.vector.tensor_scalar_mul(wtmp[0:64, :], wtmp[0:64, :], float(ALPHA))
        nc.vector.tensor_scalar_mul(wtmp[64:128, :], wtmp[64:128, :], float(ALPHA * C3))
        wtmp2 = wpool.tile([P, 64], f32, tag="wtmp2")
        nc.sync.dma_start(wtmp2[0:64, :], w3_d[:, :])
        nc.sync.dma_start(wtmp2[64:128, :], w4_d[:, :])
        nc.vector.tensor_scalar_mul(wtmp2[:, :], wtmp2[:, :], float(ALPHA * C3))

        LA = wpool.tile([P, 128], f16, tag="LA")  # [W1'; 0]
        LB = wpool.tile([P, 128], f16, tag="LB")  # [0; W2']
        LC = wpool.tile([P, 128], f16, tag="LC")  # [W2'; W2']
        LD = wpool.tile([P, 128], f16, tag="LD")  # [[0|W1'c64]; [W4'|0]]
        LE = wpool.tile([P, 128], f16, tag="LE")  # [[W3'|0]; 0]
        LF = wpool.tile([P, 128], f16, tag="LF")  # blockdiag(W4', W4')
        LG = wpool.tile([P, 128], f16, tag="LG")  # blockdiag(W3', W3')
        for L in (LA, LB, LD, LE, LF, LG):
            nc.vector.memzero(L[:, :])
        nc.scalar.copy(LA[0:64, :], wtmp[0:64, :])
        nc.scalar.copy(LB[64:128, :], wtmp[64:128, :])
        nc.scalar.copy(LC[0:64, :], wtmp[64:128, :])
        nc.scalar.copy(LC[64:128, :], wtmp[64:128, :])
        nc.scalar.copy(LD[64:128, 0:64], wtmp2[64:128, :])
        nc.scalar.copy(LD[0:64, 64:128], wtmp[0:64, 0:64])
        nc.scalar.copy(LE[0:64, 0:64], wtmp2[0:64, :])
        nc.scalar.copy(LF[0:64, 0:64], wtmp2[64:128, :])
        nc.scalar.copy(LF[64:128, 64:128], wtmp2[64:128, :])
        nc.scalar.copy(LG[0:64, 0:64], wtmp2[0:64, :])
        nc.scalar.copy(LG[64:128, 64:128], wtmp2[0:64, :])

        # whole a0 row resident on partition 0 (51KB); one DMA for the run
        ARall = wpool.tile([1, N_PAD], f16, tag="ARall")
        nc.gpsimd.dma_start(ARall[0:1, :], arow_d[:, :])

        def prep_dma(ch):
            z0 = ch * CHUNK
            F = ft.tile([P, 3, CHUNK], f16, tag="F")
            nc.sync.dma_start(
                F[:], featsT_d[:, z0 : z0 + CHUNK].rearrange("(t p) n -> p t n", p=P)
            )
            AM = amp.tile([P, 2, CHUNK], f16, tag="AM")
            nc.gpsimd.dma_start(
                AM[:], amix_d[:, z0 : z0 + CHUNK].rearrange("(t p) n -> p t n", p=P)
            )
            A0 = a0p.tile([P, CHUNK], f16, tag="A0")
            nc.gpsimd.partition_broadcast(A0[:, :], ARall[0:1, z0 : z0 + CHUNK])
            return F, AM, A0

        def prep_prod(ch, tiles):
            F, AM, A0 = tiles
            P12 = s12p.tile([P, 2, CHUNK], f16, tag="P12")
            P45 = s45p.tile([P, 2, CHUNK], f16, tag="P45")
            P3 = s3p.tile([P, CHUNK], f16, tag="P3")
            a0b = A0[:, :].unsqueeze(1).to_broadcast((P, 2, CHUNK))
            m12b = AM[:, 1].unsqueeze(1).to_broadcast((P, 2, CHUNK))
            nc.vector.tensor_tensor(P12[:, :], F[:, 0:2], a0b, MULT)
            nc.vector.tensor_tensor(P45[:, :], F[:, 1:3], m12b, MULT)
            nc.vector.tensor_tensor(P3[:, :], F[:, 0], AM[:, 0], MULT)
            return P12, P45, P3

        def crunch_mm(ch, tiles):
            P12, P45, P3 = tiles
            B1 = psb.tile([P, CHUNK], f32, tag="B1")  # [s; g]
            PSX = psx.tile([P, 2, CHUNK], f32, tag="PSX")  # [o1_0;junk | o1_1;o1_2]
            plan = [
                (LA, B1, None, P12, 0, True, False),
                (LB, B1, None, P3, None, False, False),
                (LC, B1, None, P45, 0, False, True),
                (LD, PSX, 0, P12, 0, True, False),
                (LE, PSX, 0, P3, None, False, True),
                (LF, PSX, 1, P12, 1, True, False),
                (LG, PSX, 1, P45, 1, False, True),
            ]
            for L, dst, dcol, src, scol, st, sp in plan:
                for h in range(2):
                    hs = slice(h * HALF, (h + 1) * HALF)
                    d = dst[:, hs] if dcol is None else dst[:, dcol, hs]
                    s = src[:, hs] if scol is None else src[:, scol, hs]
                    nc.tensor.matmul(d, L[:, :], s, start=st, stop=sp)

            # tanh in the matmul stage: U is ready before the gate stage.
            # sigmoid(g) = 0.5*tanh(g/2)+0.5 -- tanh and silu share one ACT
            # table set (sigmoid does not); the affine folds into the gating.
            U = up.tile([P, CHUNK], f16, tag="U")  # [th; th], th = tanh(g/2)
            nc.scalar.activation(U[0:64, :], B1[64:128, :], AF.Tanh, scale=0.5)
            nc.scalar.activation(U[64:128, :], B1[64:128, :], AF.Tanh, scale=0.5)
            return B1, PSX, U

        def crunch_gate(ch, tiles):
            B1, PSX, U = tiles
            z0 = ch * CHUNK
            OT = otp.tile([P, 2, CHUNK], f16, tag="OT")
            OTS = osp.tile([64, CHUNK], f16, tag="OTS")
            ub = U[:, :].unsqueeze(1).to_broadcast((P, 2, CHUNK))
            # OT = (th*0.5 + 0.5) * PSX = sigmoid(g) * [o1_0;junk | o1_1;o1_2]
            nc.vector._custom_dve(
                AFFINE_MUL_REDUCE, out=OT[:, :], in0=ub, in1=PSX[:, :],
                s0=0.5, s1=0.5,
            )
            # silu into its own tile so it doesn't serialize behind the gating
            nc.scalar.activation(OTS[:, :], B1[0:64, :], AF.Silu)

            # outT rows: 0:64 = OT[0:64,0], 64:128 = OTS, 128:256 = OT[:,1]
            nc.sync.dma_start(outT_d[0:64, z0 : z0 + CHUNK], OT[0:64, 0])
            nc.scalar.dma_start(outT_d[64:128, z0 : z0 + CHUNK], OTS[:, :])
            nc.sync.dma_start(outT_d[128:256, z0 : z0 + CHUNK], OT[:, 1])

        # 5-stage pipeline: dma(ch) -> products(ch-2) -> matmuls(ch-3)
        # -> gate+store(ch-4). Products precede AMR in the DVE queue.
        raw = {}
        prod = {}
        mmres = {}
        for ch in range(N_CHUNKS + 4):
            if ch - 4 in mmres:
                crunch_gate(ch - 4, mmres.pop(ch - 4))
            if ch < N_CHUNKS:
                raw[ch] = prep_dma(ch)
            if ch - 2 in raw:
                prod[ch - 2] = prep_prod(ch - 2, raw.pop(ch - 2))
            if ch - 3 in prod:
                mmres[ch - 3] = crunch_mm(ch - 3, prod.pop(ch - 3))

        for pool in (
            psx, psb, osp, otp, up, s3p, s45p, s12p, a0p, amp, ft, wpool
        ):
            pool.release()

    nc.compile()
    return nc


def _get_nc():
    global _BUILT
    if _BUILT is None:
        _BUILT = _build_nc()
    return _BUILT


def _host_prep(node_feats, node_attrs):
    """Feature-major fp16 layouts per core (layout/dtype/replication only)."""
    feats = np.ascontiguousarray(node_feats, dtype=np.float32)
    attrs = np.ascontiguousarray(node_attrs, dtype=np.float32)
    in_maps = []
    for c in range(N_CORES):
        f = feats[c * N_PER : (c + 1) * N_PER]
        a = attrs[c * N_PER : (c + 1) * N_PER].astype(np.float16)
        n = f.shape[0]
        x0 = f[:, :MUL].astype(np.float16).T
        x1 = f[:, MUL:].reshape(n, MUL, 3).astype(np.float16)
        ftT = np.zeros((384, N_PAD), np.float16)
        ftT[0:64, :n] = x0
        ftT[64:128, :n] = x1[:, :, 0].T
        ftT[128:192, :n] = x1[:, :, 1].T
        ftT[192:256, :n] = x1[:, :, 2].T
        ftT[256:320, :n] = x0
        ftT[320:384, :n] = x0
        amix = np.zeros((256, N_PAD), np.float16)
        amix[0:128, :n] = a[:, 1][None, :]  # a1_0 replicated
        amix[128:192, :n] = a[:, 2][None, :]  # a1_1
        amix[192:256, :n] = a[:, 3][None, :]  # a1_2
        arow = np.zeros((1, N_PAD), np.float16)
        arow[0, :n] = a[:, 0]  # a0
        in_maps.append({"featsT": ftT, "amix": amix, "arow": arow})
    return in_maps


def kernel(node_feats, node_attrs, W1, W2, W3, W4):
    from concourse.bass_utils import run_bass_kernel_spmd

    nc = _get_nc()
    in_maps = _host_prep(node_feats, node_attrs)
    for im in in_maps:
        im["W1"] = np.ascontiguousarray(W1, np.float32)
        im["W2"] = np.ascontiguousarray(W2, np.float32)
        im["W3"] = np.ascontiguousarray(W3, np.float32)
        im["W4"] = np.ascontiguousarray(W4, np.float32)

    res = run_bass_kernel_spmd(nc, in_maps, list(range(N_CORES)))
    global LAST_RESULT
    LAST_RESULT = res

    outs = []
    for c in range(N_CORES):
        oT = res.results[c]["outT"][:, :N_PER].astype(np.float32)  # [256, n]
        n = oT.shape[1]
        out = np.empty((n, 256), np.float32)
        # outT rows: 0:64 = gated o1_0, 64:128 = silu(s),
        #            128:192 = gated o1_1, 192:256 = gated o1_2
        out[:, :MUL] = oT[64:128, :].T
        g = np.stack([oT[0:64], oT[128:192], oT[192:256]], axis=0)  # [3, 64, n]
        out[:, MUL:] = g.transpose(2, 1, 0).reshape(n, 192)
        outs.append(out)
    return np.concatenate(outs, axis=0)


LAST_RESULT = None


# revision 30
# speedup vs baseline: 1.1632x; 1.0377x over previous
"""Trainium2 Bass kernel for the gated equivariant tensor-product layer.

Math (per node z, MUL=64):
  x0 = feats[:, :64], x1[u,i] = feats[:, 64+3u+i], a0 = attrs[:,0], a1 = attrs[:,1:4]
  out0 = ALPHA*( (x0*a0) @ W1 + C*(sum_i x1_i*a1_i) @ W2 )          # [N,128] = s|g
  out1_i = ALPHA*C*( (x0*a1_i) @ W3 + (x1_i*a0) @ W4 )              # [N,64] per i
  out = [ silu(s) | sigmoid(g)[w]*out1_i[w] at col 64+3w+i ]

Design v6 (engine-balanced; elementwise on DVE at fp16 2x-mode):
 - HOST layouts (layout/dtype/replication only): featsT [384,n] =
   [T1|T2|T3], T1=[x0;x1_0], T2=[x1_1;x1_2], T3=[x0;x0]; amix [256,n] =
   [a1_0-replicated | a1_1;a1_2 replicated]; arow [1,n] = a0.
 - arow preloaded whole (51KB, partition 0); per chunk one gpsimd
   partition_broadcast builds A0 [128,n] (a0); amix/featsT stream in.
 - DVE forms 5 product tiles in 3 fused ops (3D APs + free-dim stride-0
   broadcast on in1, all fp16 SBUF -> DVE 2x mode):
     P12 = [T1|T2] * bcast(A0)  -> [t0;t4_0 | t4_1;t4_2]
     P45 = [T2|T3] * bcast(M12) -> [dt_1;dt_2 | t3_1;t3_2]
     P3  =  T1     * A10        -> [t3_0;dt_0]
 - 7 logical fp16 matmuls with zero-padded/blockdiag/fused-column lhsT:
     B1 = LA'P1+LB'P3+LC'P4 = [s;g]; PSX col0 = LD'P1+LE'P3 = [o1_0;junk]
     PSX col1 = LF'P2+LG'P5 = [o1_1;o1_2]  (junk = defined filler)
 - sigmoid via tanh: sigmoid(g)=0.5*tanh(g/2)+0.5 so Tanh+Silu share ONE
   ACT table set (no per-chunk table reloads). ACT: tanh twice ->
   U=[th;th]; gating = single custom-DVE AFFINE_MUL_REDUCE
   (th*0.5+0.5)*PSX over [128,2,n] with free-dim-broadcast U; ACT Silu(s)
   writes its own tile (3 output stores; junk slot never stored).
 - 5-stage software pipeline emitted gate-first so the gating op heads
   the DVE queue: gate(ch-4) | dma(ch) | products(ch-2) | matmuls(ch-3).

Sharding: pure data parallelism over nodes, 8 cores x 25000 nodes
(padded to 25600 = 25 chunks of 1024 per core).
"""

import sys
import numpy as np

sys.path.insert(0, "/opt/trn_rl_repo")

MUL = 64
C3 = 1.0 / np.sqrt(3.0)
ALPHA = 1.0 / np.sqrt(MUL * 1 * 2)

N_CORES = 8
N_PER = 25000
N_PAD = 25600
CHUNK = 1024
HALF = CHUNK // 2
N_CHUNKS = N_PAD // CHUNK
P = 128

_BUILT = None


def _build_nc():
    import concourse.bacc as bacc
    import concourse.mybir as mybir
    from concourse.tile import TileContext
    from concourse.dve_ops import AFFINE_MUL_REDUCE

    f32 = mybir.dt.float32
    f16 = mybir.dt.float16
    MULT = mybir.AluOpType.mult
    AF = mybir.ActivationFunctionType

    nc = bacc.Bacc("TRN2", target_bir_lowering=False, debug=False)

    featsT_d = nc.declare_dram_parameter("featsT", [384, N_PAD], f16, isOutput=False)
    # amix rows: 0:128 = a1_0 replicated, 128:256 = [a1_1; a1_2] repl x64
    amix_d = nc.declare_dram_parameter("amix", [256, N_PAD], f16, isOutput=False)
    arow_d = nc.declare_dram_parameter("arow", [1, N_PAD], f16, isOutput=False)
    w1_d = nc.declare_dram_parameter("W1", [64, 128], f32, isOutput=False)
    w2_d = nc.declare_dram_parameter("W2", [64, 128], f32, isOutput=False)
    w3_d = nc.declare_dram_parameter("W3", [64, 64], f32, isOutput=False)
    w4_d = nc.declare_dram_parameter("W4", [64, 64], f32, isOutput=False)
    outT_d = nc.declare_dram_parameter("outT", [256, N_PAD], f16, isOutput=True)

    with TileContext(nc) as tc:
        wpool = tc.alloc_tile_pool(name="wpool", bufs=1)
        ft = tc.alloc_tile_pool(name="ft", bufs=5)
        amp = tc.alloc_tile_pool(name="amp", bufs=5)
        a0p = tc.alloc_tile_pool(name="a0p", bufs=5)
        s12p = tc.alloc_tile_pool(name="s12p", bufs=2)
        s45p = tc.alloc_tile_pool(name="s45p", bufs=2)
        s3p = tc.alloc_tile_pool(name="s3p", bufs=2)
        up = tc.alloc_tile_pool(name="up", bufs=2)
        otp = tc.alloc_tile_pool(name="otp", bufs=3)
        osp = tc.alloc_tile_pool(name="osp", bufs=3)
        psb = tc.alloc_tile_pool(name="psb", bufs=2, space="PSUM")
        psx = tc.alloc_tile_pool(name="psx", bufs=1, space="PSUM")

        # ---- weights (once): build 7 lhsT tiles [128,128] f16 ----
        # W1' = ALPHA*W1, W2' = ALPHA*C3*W2, W3' = ALPHA*C3*W3, W4' = ALPHA*C3*W4
        wtmp = wpool.tile([P, 128], f32, tag="wtmp")
        nc.sync.dma_start(wtmp[0:64, :], w1_d[:, :])
        nc.sync.dma_start(wtmp[64:128, :], w2_d[:, :])
        nc.vector.tensor_scalar_mul(wtmp[0:64, :], wtmp[0:64, :], float(ALPHA))
        nc.vector.tensor_scalar_mul(wtmp[64:128, :], wtmp[64:128, :], float(ALPHA * C3))
        wtmp2 = wpool.tile([P, 64], f32, tag="wtmp2")
        nc.sync.dma_start(wtmp2[0:64, :], w3_d[:, :])
        nc.sync.dma_start(wtmp2[64:128, :], w4_d[:, :])
        nc.vector.tensor_scalar_mul(wtmp2[:, :], wtmp2[:, :], float(ALPHA * C3))

        LA = wpool.tile([P, 128], f16, tag="LA")  # [W1'; 0]
        LB = wpool.tile([P, 128], f16, tag="LB")  # [0; W2']
        LC = wpool.tile([P, 128], f16, tag="LC")  # [W2'; W2']
        LD = wpool.tile([P, 128], f16, tag="LD")  # [[0|W1'c64]; [W4'|0]]
        LE = wpool.tile([P, 128], f16, tag="LE")  # [[W3'|0]; 0]
        LF = wpool.tile([P, 128], f16, tag="LF")  # blockdiag(W4', W4')
        LG = wpool.tile([P, 128], f16, tag="LG")  # blockdiag(W3', W3')
        for L in (LA, LB, LD, LE, LF, LG):
            nc.vector.memzero(L[:, :])
        nc.scalar.copy(LA[0:64, :], wtmp[0:64, :])
        nc.scalar.copy(LB[64:128, :], wtmp[64:128, :])
        nc.scalar.copy(LC[0:64, :], wtmp[64:128, :])
        nc.scalar.copy(LC[64:128, :], wtmp[64:128, :])
        nc.scalar.copy(LD[64:128, 0:64], wtmp2[64:128, :])
        nc.scalar.copy(LD[0:64, 64:128], wtmp[0:64, 0:64])
        nc.scalar.copy(LE[0:64, 0:64], wtmp2[0:64, :])
        nc.scalar.copy(LF[0:64, 0:64], wtmp2[64:128, :])
        nc.scalar.copy(LF[64:128, 64:128], wtmp2[64:128, :])
        nc.scalar.copy(LG[0:64, 0:64], wtmp2[0:64, :])
        nc.scalar.copy(LG[64:128, 64:128], wtmp2[0:64, :])

        # whole a0 row resident on partition 0 (51KB); one DMA for the run
        ARall = wpool.tile([1, N_PAD], f16, tag="ARall")
        nc.gpsimd.dma_start(ARall[0:1, :], arow_d[:, :])

        def prep_dma(ch):
            z0 = ch * CHUNK
            F = ft.tile([P, 3, CHUNK], f16, tag="F")
            nc.sync.dma_start(
                F[:], featsT_d[:, z0 : z0 + CHUNK].rearrange("(t p) n -> p t n", p=P)
            )
            AM = amp.tile([P, 2, CHUNK], f16, tag="AM")
            nc.gpsimd.dma_start(
                AM[:], amix_d[:, z0 : z0 + CHUNK].rearrange("(t p) n -> p t n", p=P)
            )
            A0 = a0p.tile([P, CHUNK], f16, tag="A0")
            nc.gpsimd.partition_broadcast(A0[:, :], ARall[0:1, z0 : z0 + CHUNK])
            return F, AM, A0

        def prep_prod(ch, tiles):
            F, AM, A0 = tiles
            P12 = s12p.tile([P, 2, CHUNK], f16, tag="P12")
            P45 = s45p.tile([P, 2, CHUNK], f16, tag="P45")
            P3 = s3p.tile([P, CHUNK], f16, tag="P3")
            a0b = A0[:, :].unsqueeze(1).to_broadcast((P, 2, CHUNK))
            m12b = AM[:, 1].unsqueeze(1).to_broadcast((P, 2, CHUNK))
            nc.vector.tensor_tensor(P12[:, :], F[:, 0:2], a0b, MULT)
            nc.vector.tensor_tensor(P45[:, :], F[:, 1:3], m12b, MULT)
            nc.vector.tensor_tensor(P3[:, :], F[:, 0], AM[:, 0], MULT)
            return P12, P45, P3

        def crunch_mm(ch, tiles):
            P12, P45, P3 = tiles
            B1 = psb.tile([P, CHUNK], f32, tag="B1")  # [s; g]
            PSX = psx.tile([P, 2, CHUNK], f32, tag="PSX")  # [o1_0;junk | o1_1;o1_2]
            plan = [
                (LA, B1, None, P12, 0, True, False),
                (LB, B1, None, P3, None, False, False),
                (LC, B1, None, P45, 0, False, True),
                (LD, PSX, 0, P12, 0, True, False),
                (LE, PSX, 0, P3, None, False, True),
                (LF, PSX, 1, P12, 1, True, False),
                (LG, PSX, 1, P45, 1, False, True),
            ]
            for L, dst, dcol, src, scol, st, sp in plan:
                for h in range(2):
                    hs = slice(h * HALF, (h + 1) * HALF)
                    d = dst[:, hs] if dcol is None else dst[:, dcol, hs]
                    s = src[:, hs] if scol is None else src[:, scol, hs]
                    nc.tensor.matmul(d, L[:, :], s, start=st, stop=sp)

            # tanh in the matmul stage: U is ready before the gate stage.
            # sigmoid(g) = 0.5*tanh(g/2)+0.5 -- tanh and silu share one ACT
            # table set (sigmoid does not); the affine folds into the gating.
            U = up.tile([P, CHUNK], f16, tag="U")  # [th; th], th = tanh(g/2)
            nc.scalar.activation(U[0:64, :], B1[64:128, :], AF.Tanh, scale=0.5)
            nc.scalar.activation(U[64:128, :], B1[64:128, :], AF.Tanh, scale=0.5)
            return B1, PSX, U

        def crunch_gate(ch, tiles):
            B1, PSX, U = tiles
            z0 = ch * CHUNK
            OT = otp.tile([P, 2, CHUNK], f16, tag="OT")
            OTS = osp.tile([64, CHUNK], f16, tag="OTS")
            ub = U[:, :].unsqueeze(1).to_broadcast((P, 2, CHUNK))
            # OT = (th*0.5 + 0.5) * PSX = sigmoid(g) * [o1_0;junk | o1_1;o1_2]
            nc.vector._custom_dve(
                AFFINE_MUL_REDUCE, out=OT[:, :], in0=ub, in1=PSX[:, :],
                s0=0.5, s1=0.5,
            )
            # silu into its own tile so it doesn't serialize behind the gating
            nc.scalar.activation(OTS[:, :], B1[0:64, :], AF.Silu)

            # outT rows: 0:64 = OT[0:64,0], 64:128 = OTS, 128:256 = OT[:,1]
            nc.sync.dma_start(outT_d[0:64, z0 : z0 + CHUNK], OT[0:64, 0])
            nc.scalar.dma_start(outT_d[64:128, z0 : z0 + CHUNK], OTS[:, :])
            nc.sync.dma_start(outT_d[128:256, z0 : z0 + CHUNK], OT[:, 1])

        # 5-stage pipeline: dma(ch) -> products(ch-2) -> matmuls(ch-3)
        # -> gate+store(ch-4). Products precede AMR in the DVE queue.
        raw = {}
        prod = {}
        mmres = {}
        for ch in range(N_CHUNKS + 4):
            if ch - 4 in mmres:
                crunch_gate(ch - 4, mmres.pop(ch - 4))
            if ch < N_CHUNKS:
                raw[ch] = prep_dma(ch)
            if ch - 2 in raw:
                prod[ch - 2] = prep_prod(ch - 2, raw.pop(ch - 2))
            if ch - 3 in prod:
                mmres[ch - 3] = crunch_mm(ch - 3, prod.pop(ch - 3))

        for pool in (
            psx, psb, osp, otp, up, s3p, s45p, s12p, a0p, amp, ft, wpool
        ):
            pool.release()

    nc.compile()
    return nc


def _get_nc():
    global _BUILT
    if _BUILT is None:
        _BUILT = _build_nc()
    return _BUILT


def _host_prep(node_feats, node_attrs):
    """Feature-major fp16 layouts per core (layout/dtype/replication only)."""
    feats = np.ascontiguousarray(node_feats, dtype=np.float32)
    attrs = np.ascontiguousarray(node_attrs, dtype=np.float32)
    in_maps = []
    for c in range(N_CORES):
        f = feats[c * N_PER : (c + 1) * N_PER]
        a = attrs[c * N_PER : (c + 1) * N_PER].astype(np.float16)
        n = f.shape[0]
        x0 = f[:, :MUL].astype(np.float16).T
        x1 = f[:, MUL:].reshape(n, MUL, 3).astype(np.float16)
        ftT = np.zeros((384, N_PAD), np.float16)
        ftT[0:64, :n] = x0
        ftT[64:128, :n] = x1[:, :, 0].T
        ftT[128:192, :n] = x1[:, :, 1].T
        ftT[192:256, :n] = x1[:, :, 2].T
        ftT[256:320, :n] = x0
        ftT[320:384, :n] = x0
        amix = np.zeros((256, N_PAD), np.float16)
        amix[0:128, :n] = a[:, 1][None, :]  # a1_0 replicated
        amix[128:192, :n] = a[:, 2][None, :]  # a1_1
        amix[192:256, :n] = a[:, 3][None, :]  # a1_2
        arow = np.zeros((1, N_PAD), np.float16)
        arow[0, :n] = a[:, 0]  # a0
        in_maps.append({"featsT": ftT, "amix": amix, "arow": arow})
    return in_maps


def kernel(node_feats, node_attrs, W1, W2, W3, W4):
    from concourse.bass_utils import run_bass_kernel_spmd

    nc = _get_nc()
    in_maps = _host_prep(node_feats, node_attrs)
    for im in in_maps:
        im["W1"] = np.ascontiguousarray(W1, np.float32)
        im["W2"] = np.ascontiguousarray(W2, np.float32)
        im["W3"] = np.ascontiguousarray(W3, np.float32)
        im["W4"] = np.ascontiguousarray(W4, np.float32)

    res = run_bass_kernel_spmd(nc, in_maps, list(range(N_CORES)))
    global LAST_RESULT
    LAST_RESULT = res

    outs = []
    for c in range(N_CORES):
        oT = res.results[c]["outT"][:, :N_PER].astype(np.float32)  # [256, n]
        n = oT.shape[1]
        out = np.empty((n, 256), np.float32)
        # outT rows: 0:64 = gated o1_0, 64:128 = silu(s),
        #            128:192 = gated o1_1, 192:256 = gated o1_2
        out[:, :MUL] = oT[64:128, :].T
        g = np.stack([oT[0:64], oT[128:192], oT[192:256]], axis=0)  # [3, 64, n]
        out[:, MUL:] = g.transpose(2, 1, 0).reshape(n, 192)
        outs.append(out)
    return np.concatenate(outs, axis=0)


LAST_RESULT = None
